# revision 46
# baseline (speedup 1.0000x reference)
"""IterSpatialCorrelationSampler (P=9, DP=1) Trainium2 Bass kernel.

out[b,i,j,y,x] = sum_c in1[b,c,y,x] * pad(in2)[b,c,y+i,x+j]   (pad=4 each side)

Strategy (v2):
  - 8 cores, each handles (b, yhalf): b = core//2, 48 rows of y.
  - TensorE Gram-band formulation: m-tile = 8y x 16x = 128 output positions
    (PSUM partitions), n = 16x24 = 384 window of padded in2 (free dim),
    contraction over c (256 = 2 accumulating matmuls of k=128).
    psum[(yt,xt), (dy,dx)] = sum_c in1[c, y0+yt, x0+xt] * in2pad[c, y0+dy, x0+dx]
    The 81 useful values per position are psum[(yt,xt), (yt+di, xt+dj)].
  - The matmul moving operand is a strided 2D window AP directly into the
    compact padded in2 SBUF tile (no window materialization copies).
  - PSUM -> SBUF evacuation alternates ACT/DVE, full 384/partition (1 op).
  - Band DMA-out is row-extracted: for partition group g (yt=g, 16
    partitions), only window rows g..g+8 (216 contiguous elems) are
    stored: 8 sliced DMAs per ty -> 2.65 MB instead of 4.72 MB.
  - Host extracts the 81 (di,dj) diagonals from the row-extracted band.
  - PE warm-up: dummy matmuls at kernel start keep the PE busy while the
    first DMAs land, flipping the HAM clock gate to 2.4 GHz early.
  - Inputs cast to fp16 on host. PSUM accumulation is fp32.
"""

import numpy as np

import concourse.bass as bass
import concourse.bacc as bacc
import concourse.tile as tile
import concourse.mybir as mybir
from concourse.bass_utils import run_bass_kernel_spmd

# problem constants (hardcoded per contract)
B, C, H, W = 4, 256, 96, 128
P = 9
OFF = 4
NCORES = 8
YH = H // 2          # 48 rows per core
WP = W + 2 * OFF     # 136
ROWS = YH + 2 * OFF  # 56 rows of padded in2 per core
MT_Y, MT_X = 8, 16   # m-tile shape (8y x 16x = 128 partitions)
NW_Y, NW_X = MT_Y + P - 1, MT_X + P - 1   # 16 x 24 window
NTY, NTX = YH // MT_Y, W // MT_X          # 6 x 8 = 48 tiles
NFREE = NW_Y * NW_X                       # 384
RE = P * NW_X                             # 216 row-extracted elems/partition
NWARM = 8                                 # PE warm-up dummy matmuls
NFILL = {3: 4, 4: 6, 5: 6}                # PE filler matmuls at late-ty stalls
WAVES = [(0, 2), (2, 4), (4, 5), (5, 6)]  # band store waves (ty ranges)

_cached = {}


def _build():
    nc = bacc.Bacc(
        "TRN2",
        target_bir_lowering=False,
        debug=False,
        enable_asserts=False,
        num_devices=NCORES,
    )
    f16 = mybir.dt.float16
    f32 = mybir.dt.float32

    in1_d = nc.dram_tensor(
        "in1t", [128, NTY, NTX, 2, MT_Y * MT_X], f16, kind="ExternalInput"
    ).ap()
    in2_d = nc.dram_tensor("in2c", [128, 2, ROWS, WP], f16, kind="ExternalInput").ap()
    # [g, lp, ty, di(9 rows), tx, wx] — dim order matches the SBUF source
    # [lp-partitions, ty, rows...] so wave stores are plain 3-dim APs
    band_d = nc.dram_tensor(
        "rband", [MT_Y, NW_Y, NTY, P, NTX, NW_X], f16, kind="ExternalOutput"
    ).ap()

    with tile.TileContext(nc) as tc:
        with (
            tc.tile_pool(name="sb2", bufs=1) as sb2,
            tc.tile_pool(name="ld", bufs=5) as ld,
            tc.tile_pool(name="stage", bufs=3) as stage,
            tc.tile_pool(name="warm", bufs=1) as warm,
            tc.tile_pool(name="ps", bufs=3, space="PSUM") as ps,
            tc.tile_pool(name="psw", bufs=1, space="PSUM") as psw,
        ):
            in2_sb = sb2.tile([128, 2, ROWS, WP], f16)
            # whole-run band staging buffer: [p, ty, wy, tx, wx]; the
            # row-extracted slice (rows g..g+8, all tx, a ty-range) is a
            # 3-dim DMA AP with 1728-elem contiguous runs
            bs = sb2.tile([128, NTY, NW_Y, NTX, NW_X], f16)
            in1_cs = [
                ld.tile([128, NTX, 2, MT_Y * MT_X], f16, tag="in1c", name=f"in1c{i}")
                for i in range(2)
            ]

            # PE warm-up: dummy matmuls keep the PE active while input DMAs
            # land (HAM flips to 2.4 GHz after ~3.4us of sustained activity).
            # Specific warmups read the SBUF regions that later loads write,
            # creating WAR deps that DELAY those loads: the first compute
            # tile's inputs (in2 rows 0:16 + in1 ty0) get the HBM wire to
            # themselves, pulling the first real matmul earlier.
            ws = warm.tile([128, 512], f16)
            nc.vector.memset(ws[:, :], 0.0)
            wp = psw.tile([128, 512], f32)
            for _ in range(NWARM):
                nc.tensor.matmul(wp[:, :], ws[:, 0:128], ws[:, :], start=True, stop=True)

            # load priority: first compute tile's deps first (ty0+chunk0);
            # the rest are WAR-gated behind warmup matmuls (see gate_rhs)
            nc.sync.dma_start(out=in2_sb[:, :, 0:16, :], in_=in2_d[:, :, 0:16, :])
            nc.sync.dma_start(out=in1_cs[0][:, :, :, :], in_=in1_d[:, 0, :, :, :])
            nc.sync.dma_start(out=in1_cs[1][:, :, :, :], in_=in1_d[:, 1, :, :, :])
            nc.sync.dma_start(out=in2_sb[:, :, 16:32, :], in_=in2_d[:, :, 16:32, :])
            nc.sync.dma_start(out=in2_sb[:, :, 32:ROWS, :], in_=in2_d[:, :, 32:ROWS, :])

            for ty in range(NTY):
                if ty >= 2:
                    in1_c = ld.tile([128, NTX, 2, MT_Y * MT_X], f16, tag="in1c")
                    nc.sync.dma_start(out=in1_c[:, :, :, :], in_=in1_d[:, ty, :, :, :])
                else:
                    in1_c = in1_cs[ty]
                # filler matmuls: keep PE activity up while waiting for
                # late in1 chunks, so the HAM clock gate stays at 2.4 GHz
                for _ in range(NFILL.get(ty, 0)):
                    nc.tensor.matmul(
                        wp[:, :], ws[:, 0:128], ws[:, :], start=True, stop=True
                    )
                for txp in range(NTX // 2):
                    # two tx tiles share one 2-bank PSUM tile so they can be
                    # evacuated with a single (cheaper per element) copy
                    pt2 = ps.tile([128, 2, 512], f32, tag="pt2")
                    for half in range(2):
                        tx = 2 * txp + half
                        for ch in range(2):
                            nc.tensor.matmul(
                                pt2[:, half, 0:NFREE],
                                in1_c[:, tx, ch, :],
                                in2_sb[
                                    :, ch,
                                    MT_Y * ty : MT_Y * ty + NW_Y,
                                    MT_X * tx : MT_X * tx + NW_X,
                                ],
                                start=(ch == 0),
                                stop=(ch == 1),
                            )
                    if ty == NTY - 1:
                        # last ty: two smaller evacs per pair so the final
                        # evac (which gates the tail stores) lands sooner
                        for half in range(2):
                            tx = 2 * txp + half
                            dst1 = bs[:, ty, :, tx, :]
                            src1 = pt2[:, half, 0:NFREE]
                            if (txp + half) % 2 == 0:
                                nc.scalar.mul(dst1, src1, 1.0)
                            else:
                                nc.vector.tensor_copy(dst1, src1)
                    else:
                        # dst dims [p, wy, tx, wx] -> iterate as [p, tx, wy, wx]
                        dst = bs[:, ty, :, 2 * txp : 2 * txp + 2, :].transpose(
                            [0, 2, 1, 3]
                        )
                        src = pt2[:, :, 0:NFREE]
                        if txp % 2 == 0:
                            nc.scalar.mul(dst, src, 1.0)
                        else:
                            nc.vector.tensor_copy(dst, src)
                # band store waves: one DMA per group g covers the wave's ty
                # range; issue is spread over scalar/sync/gpsimd (flat ~600ns
                # per dma_start on the issuing engine)
                # engine choice: keep compute-dependent stores out of the
                # FIFO of any engine that still has critical work queued
                # (sync is done issuing loads by ty4; ACT must finish ty5
                # evacs before its share of the tail wave).
                for w, (t0, t1) in enumerate(WAVES):
                    if ty != t1 - 1:
                        continue
                    for g in range(MT_Y):
                        if w <= 1:
                            eng = nc.gpsimd
                        elif w == 2:
                            eng = nc.gpsimd if g % 2 == 0 else nc.scalar
                        else:
                            eng = nc.scalar if g % 2 == 0 else nc.sync
                        eng.dma_start(
                            out=band_d[g, :, t0:t1, :, :, :],
                            in_=bs[g * 16 : (g + 1) * 16, t0:t1, g : g + P, :, :],
                        )

    nc.compile()
    return nc


def _prep_inputs(input1, input2):
    """Build per-core input maps (fp16, padded, tiled, c split on partitions)."""
    in_maps = []
    pad2 = np.pad(
        np.asarray(input2), ((0, 0), (0, 0), (OFF, OFF), (OFF, OFF))
    )  # [B, C, H+8, WP]
    a1 = np.asarray(input1)
    for core in range(NCORES):
        b, yh = core // 2, core % 2
        y0 = yh * YH
        # in1 tiles: [cp, ty, tx, ch, (my, mx)]
        i1 = a1[b, :, y0 : y0 + YH, :].reshape(2, 128, NTY, MT_Y, NTX, MT_X)
        i1 = i1.transpose(1, 2, 4, 0, 3, 5).reshape(128, NTY, NTX, 2, MT_Y * MT_X)
        # compact padded in2: [cp, ch, rows, cols]
        p2 = pad2[b, :, y0 : y0 + ROWS, :].reshape(2, 128, ROWS, WP)
        i2c = p2.transpose(1, 0, 2, 3).astype(np.float16)  # [128, 2, ROWS, WP]
        in_maps.append(
            {
                "in1t": np.ascontiguousarray(i1.astype(np.float16)),
                "in2c": np.ascontiguousarray(i2c),
            }
        )
    return in_maps


def _extract(rb):
    """rband [MT_Y, NW_Y, NTY, P, NTX, NW_X] f16 -> [9, 9, 48, 128].

    rb[g, lp, ty, di, tx, wx] = band value at window row (g+di), col wx
    for position (y = ty*8+g, x = tx*16+lp). Useful wx = lp + dj.
    """
    arr = rb.transpose(2, 0, 1, 3, 4, 5)  # -> [ty, g, lp, di, tx, wx]
    out = np.empty((P, P, YH, W), dtype=np.float32)
    for di in range(P):
        t = arr[:, :, :, di, :, :]  # [ty, g, lp, tx, wx]
        for dj in range(P):
            d = t.diagonal(dj, 2, 4)  # [ty, g, tx, lp(diag)]
            out[di, dj] = d.reshape(YH, W)
    return out


def run(input1, input2, trace=False, **trace_kwargs):
    if "nc" not in _cached:
        _cached["nc"] = _build()
    nc = _cached["nc"]
    in_maps = _prep_inputs(input1, input2)
    res = run_bass_kernel_spmd(
        nc, in_maps, list(range(NCORES)), trace=trace, **trace_kwargs
    )
    out = np.empty((B, P, P, H, W), dtype=np.float32)
    for core in range(NCORES):
        b, yh = core // 2, core % 2
        rb = res.results[core]["rband"]
        out[b, :, :, yh * YH : (yh + 1) * YH, :] = _extract(rb)
    return out, res


def kernel(input1, input2):
    out, _ = run(input1, input2, trace=False)
    return out
